# revision 3
# baseline (speedup 1.0000x reference)
"""Trainium2 Bass kernel for nn_CustomReshapeLayer (staircase sliding-window gather).

reference: out[b, i, j] = inputs[b, start[i] + j], start[i] = i*M - i*(i-1)//2,
shapes: inputs [32, 2098176] f32 -> out [32, 2048, 2048] f32. jnp.take's default
out-of-bounds mode fills with NaN; rows i >= ~1985 run past the end of the input,
so those positions must be NaN.

Design (data-parallel over 8 NeuronCores, 4 batch rows per core):
  - Host pads each batch row with M NaNs so out-of-bounds window tails read NaN.
  - Per 128-row block: SWDGE indirect DMA gathers 128 windows (one int32 element
    offset per partition, 8KB contiguous per window) HBM -> SBUF, then a single
    HWDGE DMA stores the [128, 2048] tile to its contiguous slot in the output.
  - Raw-Bass two-engine pipeline (gpsimd gathers / sync stores), NBUF rotating
    buffers + per-buffer semaphores. Block(no_gpsimd_drain=True) skips the
    ~35us SWDGE drain at kernel exit (all gathers are provably complete: their
    completion semaphores gate the stores, and every store is waited on).
"""

from contextlib import ExitStack

import numpy as np

import concourse.bass as bass
import concourse.mybir as mybir
from concourse.bass_utils import run_bass_kernel_spmd

M = 2048
VEC = M * (M + 1) // 2  # 2,098,176
VECP = VEC + M  # per-batch stride incl. NaN pad
B_FULL = 32
NCORES = 8
B_CORE = B_FULL // NCORES  # 4
NB = M // 128  # row-blocks per batch = 16
NGATHER = B_CORE * NB  # 64
NBUF = 8

_cache: dict = {}


def _starts() -> np.ndarray:
    i = np.arange(M, dtype=np.int64)
    return i * M - (i * (i - 1)) // 2


def _make_indices() -> np.ndarray:
    """int32 [128, NGATHER]; value (p, g) = b*VECP + start[blk*128 + p]."""
    starts = _starts()
    idx = np.empty((128, NGATHER), dtype=np.int32)
    for g in range(NGATHER):
        b, blk = divmod(g, NB)
        idx[:, g] = (b * VECP + starts[blk * 128 + np.arange(128)]).astype(np.int32)
    return idx


def _pad_input(x_core: np.ndarray) -> np.ndarray:
    """[B_CORE, VEC] -> flat [B_CORE*VECP, 1]; pad reads as jnp.take's NaN fill."""
    out = np.full((B_CORE, VECP), np.nan, dtype=np.float32)
    out[:, :VEC] = x_core
    return out.reshape(-1, 1)


def _build_nc() -> bass.Bass:
    nc = bass.Bass()
    x = nc.declare_dram_parameter(
        "x", [B_CORE * VECP, 1], mybir.dt.float32, isOutput=False
    )
    idx = nc.declare_dram_parameter(
        "idx", [128, NGATHER], mybir.dt.int32, isOutput=False
    )
    y = nc.declare_dram_parameter("y", [B_CORE, M, M], mybir.dt.float32, isOutput=True)

    with ExitStack() as stack:
        idx_sb = stack.enter_context(
            nc.sbuf_tensor("idx_sb", [128, NGATHER], mybir.dt.int32)
        )
        bufs = [
            stack.enter_context(nc.sbuf_tensor(f"buf{i}", [128, M], mybir.dt.float32))
            for i in range(NBUF)
        ]
        idx_sem = stack.enter_context(nc.semaphore("idx_sem"))
        gat_sems = [
            stack.enter_context(nc.semaphore(f"gat_sem{i}")) for i in range(NBUF)
        ]
        st_sems = [stack.enter_context(nc.semaphore(f"st_sem{i}")) for i in range(NBUF)]
        block = stack.enter_context(nc.Block(no_gpsimd_drain=True))

        @block.gpsimd
        def _(gpsimd):
            gpsimd.dma_start(out=idx_sb[:], in_=idx[:]).then_inc(idx_sem, 16)
            gpsimd.wait_ge(idx_sem, 16)
            for i in range(NGATHER):
                if i >= NBUF:
                    gpsimd.wait_ge(st_sems[i % NBUF], 16 * (i // NBUF))
                gpsimd.indirect_dma_start(
                    out=bufs[i % NBUF][:],
                    out_offset=None,
                    in_=x[:],
                    in_offset=bass.IndirectOffsetOnAxis(
                        ap=idx_sb[:, i : i + 1], axis=0
                    ),
                ).then_inc(gat_sems[i % NBUF], 16)

        @block.sync
        def _(sync):
            for i in range(NGATHER):
                sync.wait_ge(gat_sems[i % NBUF], 16 * (i // NBUF + 1))
                b, blk = divmod(i, NB)
                r0 = blk * 128
                sync.dma_start(
                    out=y[b, r0 : r0 + 128, :],
                    in_=bufs[i % NBUF][:],
                ).then_inc(st_sems[i % NBUF], 16)
            for k in range(NBUF):
                sync.wait_ge(st_sems[k], 16 * (NGATHER // NBUF))
    return nc


def _run(inputs: np.ndarray, trace: bool = False):
    """inputs [32, VEC] f32 -> (out [32, M, M] f32, exec_time_ns | None)."""
    assert inputs.shape == (B_FULL, VEC), inputs.shape
    x = np.ascontiguousarray(inputs, dtype=np.float32)
    if "nc" not in _cache:
        _cache["nc"] = _build_nc()
        _cache["idx"] = _make_indices()
    nc, idx = _cache["nc"], _cache["idx"]
    in_maps = [
        {"x": _pad_input(x[c * B_CORE : (c + 1) * B_CORE]), "idx": idx}
        for c in range(NCORES)
    ]
    res = run_bass_kernel_spmd(nc, in_maps, list(range(NCORES)), trace=trace)
    out = np.concatenate([res.results[c]["y"] for c in range(NCORES)], axis=0)
    return out, res.exec_time_ns


def kernel(inputs: np.ndarray) -> np.ndarray:
    out, _ = _run(np.asarray(inputs))
    return out
